# revision 2
# baseline (speedup 1.0000x reference)
"""Grouped-GEMM (MoE routing) kernel for TRN2, 8 NeuronCores, SPMD.

out[m] = values[m] @ combining_matrix[species_idx[m]]
  values [131072, 128] f32, species_idx [131072] i32, combining_matrix [8, 128, 256] f32

Strategy (v2 — species-parallel, fp8 input):
  - Host: route rows by species; core c gets ALL rows of species c
    (counts are 16384 +-1%, so cores stay balanced). Each core's rows are
    packed into a transposed buffer xT [128, R] (zero-padded to the max
    species count R, identical on every core -> one SPMD program).
  - Values travel as fp8 e3m4 (1 B/elem): the PE consumes an fp8 moving
    operand against bf16 stationary weights directly (both are upcast to
    fp22 internally; fp8 runs at bf16 speed without DoubleRow). Measured
    end-to-end max-rel-error 1.37e-2 vs the 2e-2 gate (bf16 path: 3.2e-3).
  - Per-core HBM traffic: 2.1 MB x (fp8) + 64 KB w (bf16) + 8.5 MB out
    (bf16) = 10.7 MB -> ~29 us at the ~360 GB/s per-core HBM limit. The
    DMA stream is the roofline; every engine stage fits under it:
    PE ~14 us, DVE cast ~16 us, ACT cast ~17 us + 1.3 us table load.
  - Device: single weight tile [128, 256]. Per 1024-col group g, half h:
    psum[h] = w[:, h*128:+128].T @ xT[:, g] via 512-col matmul chunks
    (f32 PSUM). The PSUM->SBUF bf16 drain is split across engines: h=0
    on DVE (tensor_copy), h=1 on ACT (activation copy), so the two halves
    drain in parallel. Out-DMAs flush every 2048 cols, issued by the ring
    of the engine that produced the data (SP for h=0, ACT for h=1); the
    remainder group goes LAST so the final drain piece is tiny.
  - Host: scatter outT columns back to the full [131072, 256] f32 output.
"""

import numpy as np
import ml_dtypes
from contextlib import ExitStack

import concourse.bass as bass
import concourse.mybir as mybir
import concourse.tile as tile
from concourse import bacc
from concourse.bass_utils import run_bass_kernel_spmd

M_TOTAL = 131072
D_IN = 128
N_OUT = 256
N_SPECIES = 8
N_CORES = 8
PAD = 16           # column padding granularity (rows of the sample axis)
CHUNK = 512        # matmul moving-dim chunk (PSUM bank limit)
GROUP = 1024       # PSUM tile cols (2 banks) drained by one cast
SEG = 2048         # input DMA segment (cols); first segment is 1024
F32 = mybir.dt.float32
BF16 = ml_dtypes.bfloat16
FP8 = ml_dtypes.float8_e3m4
X_DT = mybir.dt.float8e3
MM_DT = mybir.dt.bfloat16
OUT_DT = mybir.dt.bfloat16


def _segments(r_pad):
    """Input DMA segments: [1024, 2048, ..., 2048, rem]; every segment but
    the last is a multiple of GROUP so 1024-col groups never straddle one."""
    segs = []
    pos = 0
    first = min(GROUP, r_pad)
    segs.append((0, first))
    pos = first
    while r_pad - pos > SEG:
        segs.append((pos, SEG))
        pos += SEG
    if r_pad - pos > 0:
        segs.append((pos, r_pad - pos))
    return segs


def _build_nc(r_pad):
    """Build the SPMD program for one core; r_pad = padded max species count."""
    nc = bacc.Bacc("TRN2", target_bir_lowering=False, debug=False,
                   num_devices=N_CORES)
    xT = nc.dram_tensor("xT", [D_IN, r_pad], X_DT, kind="ExternalInput").ap()
    w = nc.dram_tensor("w", [D_IN, N_OUT], MM_DT, kind="ExternalInput").ap()
    outT = nc.dram_tensor("outT", [N_OUT, r_pad], OUT_DT, kind="ExternalOutput").ap()

    segs = _segments(r_pad)

    with tile.TileContext(nc) as tc, ExitStack() as ctx:
        wpool = ctx.enter_context(tc.tile_pool(name="w", bufs=1))
        xpool = ctx.enter_context(tc.tile_pool(name="x", bufs=len(segs)))
        opool = ctx.enter_context(tc.tile_pool(name="o", bufs=2))
        psA = ctx.enter_context(tc.tile_pool(name="psA", bufs=2, space="PSUM"))
        psB = ctx.enter_context(tc.tile_pool(name="psB", bufs=2, space="PSUM"))

        # weights first on the SP ring (64 KB, lands fast), then the input
        # stream: one DMA per segment, all issued up-front. The first
        # segment is a single group so compute ramps immediately.
        wt = wpool.tile([D_IN, N_OUT], MM_DT)
        nc.sync.dma_start(wt, w)

        xts = {}          # seg start -> (tile, seg start)
        for s0, sl in segs:
            xt = xpool.tile([D_IN, sl], X_DT, tag="x", name=f"x{s0}")
            xts[s0] = xt
            nc.sync.dma_start(xt, xT[:, s0:s0 + sl])

        ots = [opool.tile([128, r_pad], OUT_DT, tag="o", name=f"ot{h}")
               for h in range(2)]

        # groups of GROUP cols; the remainder group lands LAST so the final
        # cast + out-DMA piece is small (short kernel tail).
        groups = []
        for s0, sl in segs:
            g0 = 0
            while g0 < sl:
                gw = min(GROUP, sl - g0)
                groups.append((s0, g0, gw))
                g0 += gw

        pools = [psA, psB]
        q0 = [0, 0]       # per-h flushed-up-to column
        for gi, (s0, g0, gw) in enumerate(groups):
            xseg = xts[s0]
            a0 = s0 + g0              # absolute output column
            for h in range(2):
                lhsT = wt[:, h * 128:(h + 1) * 128]
                ps = pools[h].tile([128, GROUP], F32, tag="ps",
                                   name=f"ps{h}g{gi}")
                for j in range(0, gw, CHUNK):
                    cj = min(CHUNK, gw - j)
                    nc.tensor.matmul(ps[:, j:j + cj], lhsT,
                                     xseg[:, g0 + j:g0 + j + cj],
                                     start=True, stop=True)
                if h == 0:
                    nc.vector.tensor_copy(ots[0][:, a0:a0 + gw], ps[:, :gw])
                else:
                    nc.scalar.copy(ots[1][:, a0:a0 + gw], ps[:, :gw])
                # flush every 2 full groups, plus whatever remains at the end
                flush_to = a0 + gw
                if flush_to - q0[h] >= 2 * GROUP or gi == len(groups) - 1:
                    deng = nc.sync if h == 0 else nc.scalar
                    deng.dma_start(
                        outT[h * 128:(h + 1) * 128, q0[h]:flush_to],
                        ots[h][:, q0[h]:flush_to])
                    q0[h] = flush_to

    nc.compile()
    return nc


def _prepare(values, species_idx, combining_matrix):
    """Host routing + packing. Returns (in_maps, plan)."""
    values = np.ascontiguousarray(values, dtype=np.float32)
    species_idx = np.asarray(species_idx, dtype=np.int32)
    w = np.asarray(combining_matrix, dtype=np.float32)

    rows = [np.nonzero(species_idx == c)[0] for c in range(N_CORES)]
    counts = [r.size for r in rows]
    r_pad = -(-max(max(counts), GROUP) // PAD) * PAD

    in_maps = []
    for c in range(N_CORES):
        xT = np.zeros((D_IN, r_pad), dtype=FP8)
        n = counts[c]
        if n:
            xT[:, :n] = values[rows[c]].astype(FP8).T
        in_maps.append({"xT": xT, "w": np.ascontiguousarray(w[c].astype(BF16))})

    plan = {"rows": rows, "counts": counts, "r_pad": r_pad}
    return in_maps, plan


def _postprocess(results, plan):
    rows, counts = plan["rows"], plan["counts"]
    out = np.empty((M_TOTAL, N_OUT), dtype=np.float32)
    for c in range(N_CORES):
        n = counts[c]
        if n:
            out[rows[c]] = results[c]["outT"][:, :n].T.astype(np.float32)
    return out


def kernel(values, species_idx, combining_matrix):
    in_maps, plan = _prepare(values, species_idx, combining_matrix)
    nc = _build_nc(plan["r_pad"])
    res = run_bass_kernel_spmd(nc, in_maps, list(range(N_CORES)))
    return _postprocess(res.results, plan)
